# revision 6
# baseline (speedup 1.0000x reference)
"""3D Haar DWT (single level) on Trainium2, data-parallel over 8 NeuronCores.

Input  x: (2, 32, 64, 128, 128) f32  -> 8 subbands, each (2, 32, 32, 64, 64).

Design (per core; 8 of the 64 (N*C) volumes each):
  For each group of 4 consecutive D-slices (= 2 output d-pairs) of a volume:
    1. one 256 KiB DMA brings the 4 slices into SBUF as (h=128, s, w),
    2. one fp32 PE matmul applies the H-axis transform to all 4 slices:
       o1[p, s*128+w] = sum_h M0s[h, p] * x[s, h, w], where M0s is the
       stacked [low; high] H matrix pre-scaled by the two deferred 1/sqrt2
       factors of the W- and D-axis Haar butterflies (exact x0.5 fold),
    3. ACT evicts PSUM->SBUF, de-interleaving even/odd W columns,
    4. DVE does the W-axis butterfly (even +- odd) then the D-axis
       butterfly (slice 2r +- slice 2r+1) -- plain adds/subs, exact fp32,
    5. one 256 KiB DMA writes the 2 finished d-pair outputs to DRAM.
"""

import os
import sys

import numpy as np

for _p in ("/opt/trn_rl_repo", "/root/.axon_site/_ro/trn_rl_repo"):
    if os.path.isdir(_p) and _p not in sys.path:
        sys.path.append(_p)

N, C, D, H, W = 2, 32, 64, 128, 128
G = N * C            # 64 independent (D, H, W) volumes
N_CORES = 8
GPC = G // N_CORES   # 8 volumes per core
R = D // 2
SLICES = 4           # D-slices per iteration (= 2 output d-pairs)
PAIRS = SLICES // 2
ITERS = D // SLICES

_CACHE = {}


def _build_program():
    import concourse.bacc as bacc
    import concourse.mybir as mybir
    import concourse.tile as tile
    from contextlib import ExitStack

    f32 = mybir.dt.float32
    add = mybir.AluOpType.add
    sub = mybir.AluOpType.subtract

    nc = bacc.Bacc(
        "TRN2",
        target_bir_lowering=False,
        debug=False,
        num_devices=N_CORES,
    )

    xd = nc.dram_tensor("x", [GPC, D, H, W], f32, kind="ExternalInput")
    m0d = nc.dram_tensor("m0s", [H, 128], f32, kind="ExternalInput")
    # y[g, r, ps, db, qs]: ps = h-band*64 + p, db = d-band, qs = w-band*64 + q
    yd = nc.dram_tensor("y", [GPC, R, 128, 2, 128], f32, kind="ExternalOutput")

    with ExitStack() as ctx:
        tc = ctx.enter_context(tile.TileContext(nc))
        const = ctx.enter_context(tc.tile_pool(name="const", bufs=1))
        m0t = const.tile([H, 128], f32, tag="m0")
        nc.sync.dma_start(m0t[:], m0d[:])

        xp = ctx.enter_context(tc.tile_pool(name="xp", bufs=5))
        p1 = ctx.enter_context(tc.tile_pool(name="p1", bufs=5, space="PSUM"))
        s1 = ctx.enter_context(tc.tile_pool(name="s1", bufs=5))
        scr = ctx.enter_context(tc.tile_pool(name="scr", bufs=2))
        s2 = ctx.enter_context(tc.tile_pool(name="s2", bufs=5))

        for g in range(GPC):
            for it in range(ITERS):
                s0 = it * SLICES
                xt = xp.tile([H, SLICES, W], f32, tag="xt")
                nc.sync.dma_start(
                    xt[:], xd[g, s0 : s0 + SLICES].rearrange("s h w -> h s w")
                )
                o1 = p1.tile([128, SLICES * W], f32, tag="o1")
                nc.tensor.matmul(
                    o1[:],
                    m0t[:],
                    xt[:].rearrange("h s w -> h (s w)"),
                    start=True,
                    stop=True,
                )

                # evict PSUM -> SBUF, de-interleaving w into (eo, j):
                # o1s free layout [s(4)][eo(2)][j(64)], written in (s, j, eo)
                # order to match o1's linear (s, w) order
                o1s = s1.tile([128, SLICES, 2, 64], f32, tag="o1s")
                nc.scalar.copy(
                    o1s[:].rearrange("p s eo j -> p s j eo"),
                    o1[:],
                )

                # W butterfly: scratch[s, wb, j] = o1s[s, 0, j] +- o1s[s, 1, j]
                sc = scr.tile([128, SLICES, 2, 64], f32, tag="scr")
                nc.vector.tensor_tensor(
                    sc[:, :, 0, :], o1s[:, :, 0, :], o1s[:, :, 1, :], add
                )
                nc.vector.tensor_tensor(
                    sc[:, :, 1, :], o1s[:, :, 0, :], o1s[:, :, 1, :], sub
                )

                # D butterfly: o2s[pr, db, qs] = sc[2pr] +- sc[2pr+1]
                scv = sc[:].rearrange("p s wb j -> p s (wb j)")   # (128, 4, 128)
                o2s = s2.tile([128, PAIRS, 2, 128], f32, tag="o2s")
                nc.vector.tensor_tensor(
                    o2s[:, :, 0, :], scv[:, 0::2, :], scv[:, 1::2, :], add
                )
                nc.vector.tensor_tensor(
                    o2s[:, :, 1, :], scv[:, 0::2, :], scv[:, 1::2, :], sub
                )

                r0 = s0 // 2
                nc.sync.dma_start(
                    yd[g, r0 : r0 + PAIRS].rearrange("r p a q -> p r a q"),
                    o2s[:],
                )

    nc.compile()
    return nc


def kernel(x, matrix_low_0, matrix_low_1, matrix_low_2,
           matrix_high_0, matrix_high_1, matrix_high_2):
    from concourse.bass_utils import run_bass_kernel_spmd

    x = np.ascontiguousarray(np.asarray(x, dtype=np.float32))
    mh0 = np.asarray(matrix_low_0, dtype=np.float32)    # (64, 128)
    mg0 = np.asarray(matrix_high_0, dtype=np.float32)   # (64, 128)
    m1l = np.asarray(matrix_low_1, dtype=np.float32)    # (128, 64)
    ml2 = np.asarray(matrix_low_2, dtype=np.float32)    # (32, 64)

    # Deferred scales: W butterfly tap (m1l[0,0]) and D butterfly tap
    # (ml2[0,0]); both 1/sqrt2 for Haar, so the fold is x0.5 (exact).
    s2 = np.float32(np.float64(m1l[0, 0]) * np.float64(ml2[0, 0]))
    m0s = np.ascontiguousarray(
        (np.concatenate([mh0, mg0], axis=0).T * s2).astype(np.float32)
    )  # (128 h, 128 p-stack)

    if "prog" not in _CACHE:
        _CACHE["prog"] = _build_program()
    nc = _CACHE["prog"]

    xg = x.reshape(G, D, H, W)
    in_maps = [
        {
            "x": np.ascontiguousarray(xg[i * GPC : (i + 1) * GPC]),
            "m0s": m0s,
        }
        for i in range(N_CORES)
    ]
    res = run_bass_kernel_spmd(nc, in_maps, list(range(N_CORES)))
    _CACHE["last_result"] = res
    y = np.concatenate([res.results[i]["y"] for i in range(N_CORES)], axis=0)
    # y: (64, 32, 128, 2, 128) = [g, r, (hb p), db, (wb q)]
    full = y.reshape(N, C, R, 2, H // 2, 2, 2, W // 2)  # n c r hb p db wb q
    out = np.transpose(full, (5, 3, 6, 0, 1, 2, 4, 7))  # db hb wb n c r p q
    out = np.ascontiguousarray(out).reshape(8, N, C, R, H // 2, W // 2)
    return tuple(out[s] for s in range(8))


# revision 7
# speedup vs baseline: 1.0228x; 1.0228x over previous
"""3D Haar DWT (single level) on Trainium2, data-parallel over 8 NeuronCores.

Input  x: (2, 32, 64, 128, 128) f32  -> 8 subbands, each (2, 32, 32, 64, 64).

Design (per core; 8 of the 64 (N*C) volumes each):
  For each group of 4 consecutive D-slices (= 2 output d-pairs) of a volume:
    1. one 256 KiB DMA brings the 4 slices into SBUF as (h=128, s, w),
    2. one fp32 PE matmul applies the H-axis transform to all 4 slices:
       o1[p, s*128+w] = sum_h M0s[h, p] * x[s, h, w], where M0s is the
       stacked [low; high] H matrix pre-scaled by the two deferred 1/sqrt2
       factors of the W- and D-axis Haar butterflies (exact x0.5 fold),
    3. ACT evicts PSUM->SBUF, de-interleaving even/odd W columns,
    4. DVE does the W-axis butterfly (even +- odd) then the D-axis
       butterfly (slice 2r +- slice 2r+1) -- plain adds/subs, exact fp32,
    5. one 256 KiB DMA writes the 2 finished d-pair outputs to DRAM.
"""

import os
import sys

import numpy as np

for _p in ("/opt/trn_rl_repo", "/root/.axon_site/_ro/trn_rl_repo"):
    if os.path.isdir(_p) and _p not in sys.path:
        sys.path.append(_p)

N, C, D, H, W = 2, 32, 64, 128, 128
G = N * C            # 64 independent (D, H, W) volumes
N_CORES = 8
GPC = G // N_CORES   # 8 volumes per core
R = D // 2
SLICES = 4           # D-slices per iteration (= 2 output d-pairs)
PAIRS = SLICES // 2
ITERS = D // SLICES

_CACHE = {}


def _build_program():
    import concourse.bacc as bacc
    import concourse.mybir as mybir
    import concourse.tile as tile
    from contextlib import ExitStack

    f32 = mybir.dt.float32
    add = mybir.AluOpType.add
    sub = mybir.AluOpType.subtract

    nc = bacc.Bacc(
        "TRN2",
        target_bir_lowering=False,
        debug=False,
        num_devices=N_CORES,
    )

    xd = nc.dram_tensor("x", [GPC, D, H, W], f32, kind="ExternalInput")
    m0d = nc.dram_tensor("m0s", [H, 128], f32, kind="ExternalInput")
    # y[g, r, ps, db, qs]: ps = h-band*64 + p, db = d-band, qs = w-band*64 + q
    yd = nc.dram_tensor("y", [GPC, R, 128, 2, 128], f32, kind="ExternalOutput")

    with ExitStack() as ctx:
        tc = ctx.enter_context(tile.TileContext(nc))
        const = ctx.enter_context(tc.tile_pool(name="const", bufs=1))
        m0t = const.tile([H, 128], f32, tag="m0")
        nc.sync.dma_start(m0t[:], m0d[:])

        xp = ctx.enter_context(tc.tile_pool(name="xp", bufs=5))
        p1 = ctx.enter_context(tc.tile_pool(name="p1", bufs=5, space="PSUM"))
        s1 = ctx.enter_context(tc.tile_pool(name="s1", bufs=5))
        scr = ctx.enter_context(tc.tile_pool(name="scr", bufs=2))
        s2 = ctx.enter_context(tc.tile_pool(name="s2", bufs=5))

        for g in range(GPC):
            for it in range(ITERS):
                s0 = it * SLICES
                xt = xp.tile([H, SLICES, W], f32, tag="xt")
                nc.sync.dma_start(
                    xt[:], xd[g, s0 : s0 + SLICES].rearrange("s h w -> h s w")
                )
                o1 = p1.tile([128, SLICES * W], f32, tag="o1")
                nc.tensor.matmul(
                    o1[:],
                    m0t[:],
                    xt[:].rearrange("h s w -> h (s w)"),
                    start=True,
                    stop=True,
                )

                # evict PSUM -> SBUF (plain contiguous copy)
                o1s = s1.tile([128, SLICES, 64, 2], f32, tag="o1s")
                nc.scalar.copy(
                    o1s[:].rearrange("p s j eo -> p (s j eo)"),
                    o1[:],
                )

                # W butterfly (stride-2 reads do the even/odd split):
                # scratch[s, wb, j] = o1s[s, j, 0] +- o1s[s, j, 1]
                sc = scr.tile([128, SLICES, 2, 64], f32, tag="scr")
                nc.vector.tensor_tensor(
                    sc[:, :, 0, :], o1s[:, :, :, 0], o1s[:, :, :, 1], add
                )
                nc.vector.tensor_tensor(
                    sc[:, :, 1, :], o1s[:, :, :, 0], o1s[:, :, :, 1], sub
                )

                # D butterfly: o2s[pr, db, qs] = sc[2pr] +- sc[2pr+1]
                scv = sc[:].rearrange("p s wb j -> p s (wb j)")   # (128, 4, 128)
                o2s = s2.tile([128, PAIRS, 2, 128], f32, tag="o2s")
                nc.vector.tensor_tensor(
                    o2s[:, :, 0, :], scv[:, 0::2, :], scv[:, 1::2, :], add
                )
                nc.vector.tensor_tensor(
                    o2s[:, :, 1, :], scv[:, 0::2, :], scv[:, 1::2, :], sub
                )

                r0 = s0 // 2
                nc.sync.dma_start(
                    yd[g, r0 : r0 + PAIRS].rearrange("r p a q -> p r a q"),
                    o2s[:],
                )

    nc.compile()
    return nc


def kernel(x, matrix_low_0, matrix_low_1, matrix_low_2,
           matrix_high_0, matrix_high_1, matrix_high_2):
    from concourse.bass_utils import run_bass_kernel_spmd

    x = np.ascontiguousarray(np.asarray(x, dtype=np.float32))
    mh0 = np.asarray(matrix_low_0, dtype=np.float32)    # (64, 128)
    mg0 = np.asarray(matrix_high_0, dtype=np.float32)   # (64, 128)
    m1l = np.asarray(matrix_low_1, dtype=np.float32)    # (128, 64)
    ml2 = np.asarray(matrix_low_2, dtype=np.float32)    # (32, 64)

    # Deferred scales: W butterfly tap (m1l[0,0]) and D butterfly tap
    # (ml2[0,0]); both 1/sqrt2 for Haar, so the fold is x0.5 (exact).
    s2 = np.float32(np.float64(m1l[0, 0]) * np.float64(ml2[0, 0]))
    m0s = np.ascontiguousarray(
        (np.concatenate([mh0, mg0], axis=0).T * s2).astype(np.float32)
    )  # (128 h, 128 p-stack)

    if "prog" not in _CACHE:
        _CACHE["prog"] = _build_program()
    nc = _CACHE["prog"]

    xg = x.reshape(G, D, H, W)
    in_maps = [
        {
            "x": np.ascontiguousarray(xg[i * GPC : (i + 1) * GPC]),
            "m0s": m0s,
        }
        for i in range(N_CORES)
    ]
    res = run_bass_kernel_spmd(nc, in_maps, list(range(N_CORES)))
    _CACHE["last_result"] = res
    y = np.concatenate([res.results[i]["y"] for i in range(N_CORES)], axis=0)
    # y: (64, 32, 128, 2, 128) = [g, r, (hb p), db, (wb q)]
    full = y.reshape(N, C, R, 2, H // 2, 2, 2, W // 2)  # n c r hb p db wb q
    out = np.transpose(full, (5, 3, 6, 0, 1, 2, 4, 7))  # db hb wb n c r p q
    out = np.ascontiguousarray(out).reshape(8, N, C, R, H // 2, W // 2)
    return tuple(out[s] for s in range(8))
